# revision 14
# baseline (speedup 1.0000x reference)
"""MoE (top-2 of 8 experts, D=1024, F=2048, T=4096) on 8 Trainium2 NeuronCores.

Strategy: expert-parallel. Every core replicates the fp32 router over all
4096 tokens, selects the tokens routed to ITS expert (top-2 membership via
max8 on logits; weights w1=sigmoid(l1-l2) renormalized pair weights),
compacts their indices with a matmul-based exclusive cumsum + indirect-DMA
scatter, gathers those token rows, runs the gated-MLP for its single expert
in bf16 (fp32 accumulate), scales each token's output row by its routing
weight, and writes a compact [CAP, D] fp32 result + the slot->token map.
The host sums the 8 compact shards into the full [T, D] output. Router
logits are computed in fp32 on-device and returned from core 0.
"""

import os
import sys

import numpy as np
import ml_dtypes

if "/opt/trn_rl_repo" not in sys.path:
    sys.path.insert(0, "/opt/trn_rl_repo")

# Problem shapes (hardcoded per contract)
T, D, F, E = 4096, 1024, 2048, 8
P = 128
NT = T // P            # 32 token tiles of 128
CAP = 1280             # per-expert token capacity (expected load 1024, ~9 sigma margin)
NJ = CAP // P          # 10 capacity tiles
GROUPS = [(0, 512), (512, 512), (1024, 256)]  # (token offset, group size) in CAP space
KO = D // P            # 8 contraction chunks over D
KI = F // P            # 16 contraction chunks over F
BIG = 100000           # position sentinel for unselected tokens
                       # (BIG * row-stride must stay well inside int32)

N_CORES = 8

_CACHE = {}


def _build_nc():
    import concourse.tile as tile
    from concourse import bacc, mybir
    from concourse.bass import IndirectOffsetOnAxis

    f32 = mybir.dt.float32
    bf = mybir.dt.bfloat16
    i32 = mybir.dt.int32
    AF = mybir.ActivationFunctionType
    AX = mybir.AxisListType
    OP = mybir.AluOpType

    nc = bacc.Bacc("TRN2", target_bir_lowering=False, debug=False,
                   enable_asserts=False, num_devices=N_CORES)

    # ---- I/O ----
    xt_d = nc.dram_tensor("x_t", [D, T], f32, kind="ExternalInput").ap()
    xb_d = nc.dram_tensor("x_bf", [T, D], bf, kind="ExternalInput").ap()
    wr_d = nc.dram_tensor("w_r", [D, E], f32, kind="ExternalInput").ap()
    wg_d = nc.dram_tensor("w_g", [D, F], bf, kind="ExternalInput").ap()
    wi_d = nc.dram_tensor("w_i", [D, F], bf, kind="ExternalInput").ap()
    wo_d = nc.dram_tensor("w_o", [F, D], bf, kind="ExternalInput").ap()
    sel_d = nc.dram_tensor("sel", [P, E], f32, kind="ExternalInput").ap()
    u128_d = nc.dram_tensor("u128", [P, P], f32, kind="ExternalInput").ap()
    u32_d = nc.dram_tensor("u32", [NT, NT], f32, kind="ExternalInput").ap()
    idf_d = nc.dram_tensor("idf", [P, P], f32, kind="ExternalInput").ap()
    idb_d = nc.dram_tensor("idb", [P, P], bf, kind="ExternalInput").ap()
    iota_d = nc.dram_tensor("iota", [P, NT], f32, kind="ExternalInput").ap()

    lg_out = nc.dram_tensor("logits_out", [T, E], f32, kind="ExternalOutput").ap()
    y_out = nc.dram_tensor("y_out", [CAP, D], f32, kind="ExternalOutput").ap()
    meta_out = nc.dram_tensor("meta_out", [CAP, 2], f32, kind="ExternalOutput").ap()
    xgath = nc.dram_tensor("xgath", [CAP, D], bf).ap()  # compacted token rows

    with tile.TileContext(nc) as tc:
        from contextlib import ExitStack
        with ExitStack() as ctx:
            consts = ctx.enter_context(tc.tile_pool(name="consts", bufs=1))
            wpool = ctx.enter_context(tc.tile_pool(name="wpool", bufs=1))
            xtp = ctx.enter_context(tc.tile_pool(name="xtp", bufs=2))
            rsm = ctx.enter_context(tc.tile_pool(name="rsm", bufs=4))
            asmp = ctx.enter_context(tc.tile_pool(name="asm", bufs=1))
            gthp = ctx.enter_context(tc.tile_pool(name="gth", bufs=2))
            xgp = ctx.enter_context(tc.tile_pool(name="xgp", bufs=2))
            mlpp = ctx.enter_context(tc.tile_pool(name="mlp", bufs=2))
            ytp = ctx.enter_context(tc.tile_pool(name="ytp", bufs=2))
            psum_mm = ctx.enter_context(tc.tile_pool(name="psmm", bufs=4, space="PSUM"))
            psum_sm = ctx.enter_context(tc.tile_pool(name="pssm", bufs=2, space="PSUM"))

            # ---- constants ----
            wr_sb = consts.tile([P, KO, E], f32)
            nc.sync.dma_start(wr_sb, wr_d.rearrange("(ko p) e -> p ko e", p=P))
            sel_sb = consts.tile([P, E], f32)
            nc.sync.dma_start(sel_sb, sel_d)
            u128_sb = consts.tile([P, P], f32)
            nc.sync.dma_start(u128_sb, u128_d)
            u32_sb = consts.tile([NT, NT], f32)
            nc.sync.dma_start(u32_sb, u32_d)
            idf_sb = consts.tile([P, P], f32)
            nc.sync.dma_start(idf_sb, idf_d)
            idb_sb = consts.tile([P, P], bf)
            nc.sync.dma_start(idb_sb, idb_d)
            iota_sb = consts.tile([P, NT], f32)
            nc.sync.dma_start(iota_sb, iota_d)

            # zero-init the compact slot map (junk slots -> token 0, weight 0)
            # and the compacted-row buffer (junk rows -> 0, not uninitialized)
            mz = asmp.tile([P, NJ, 2], f32)
            nc.vector.memset(mz, 0.0)
            nc.sync.dma_start(meta_out.rearrange("(j p) c -> p j c", p=P), mz)
            xz = asmp.tile([P, D], bf)
            nc.vector.memset(xz, 0.0)
            for j in range(NJ):
                nc.sync.dma_start(xgath[j * P:(j + 1) * P, :], xz)

            # ---- router: logits for all tokens, fp32 ----
            # logitsT[e, t-chunk] = W_router.T @ x (tiny stationary operand),
            # then PE-transpose each 128-token chunk into [t, E] layout.
            lgall = asmp.tile([P, NT, E], f32)    # logits, token t = c*128 + p
            l12 = asmp.tile([P, NT, 8], f32)      # max8 sorted logits per tile
            xt_re = xt_d.rearrange("(ko p) t -> p ko t", p=P)
            for tg in range(T // 256):
                xt_g = xtp.tile([P, KO, 256], f32, tag="xtg")
                nc.sync.dma_start(xt_g, xt_re[:, :, tg * 256:(tg + 1) * 256])
                ps_lt = psum_sm.tile([E, 256], f32, tag="small")
                for ko in range(KO):
                    nc.tensor.matmul(ps_lt, lhsT=wr_sb[:, ko], rhs=xt_g[:, ko],
                                     start=(ko == 0), stop=(ko == KO - 1))
                lgt = rsm.tile([E, 256], f32, tag="lgt")
                nc.vector.tensor_copy(lgt, ps_lt)
                for jj in range(2):
                    j = tg * 2 + jj
                    ps_l = psum_sm.tile([P, E], f32, tag="small")
                    nc.tensor.transpose(ps_l, lgt[:, jj * P:(jj + 1) * P],
                                        idf_sb[:E, :E])
                    nc.vector.tensor_copy(lgall[:, j], ps_l)
                    nc.vector.max(l12[:, j], lgall[:, j])

            # logits output (token-major [T, E])
            nc.sync.dma_start(lg_out.rearrange("(c p) e -> p c e", p=P), lgall)

            # ---- expert weight preload (bf16, stays resident). Issued after
            # the router reads so the big transfers don't delay them. ----
            wg_sb = wpool.tile([P, KO, F], bf)
            nc.sync.dma_start(wg_sb, wg_d.rearrange("(ko p) f -> p ko f", p=P))
            wi_sb = wpool.tile([P, KO, F], bf)
            nc.sync.dma_start(wi_sb, wi_d.rearrange("(ko p) f -> p ko f", p=P))
            wo_sb = wpool.tile([P, KI, D], bf)
            nc.sync.dma_start(wo_sb, wo_d.rearrange("(ki p) d -> p ki d", p=P))

            # ---- top-2 weights + this-expert membership, batched ----
            l1 = l12[:, :, 0]
            l2 = l12[:, :, 1]
            d21 = asmp.tile([P, NT], f32)
            nc.vector.tensor_sub(d21, l2, l1)
            w2a = asmp.tile([P, NT], f32)
            nc.scalar.activation(w2a, d21, AF.Sigmoid)      # w2 = sigmoid(l2 - l1)
            w1a = asmp.tile([P, NT], f32)
            nc.vector.tensor_scalar(w1a, w2a, -1.0, 1.0, op0=OP.mult, op1=OP.add)
            msel = asmp.tile([P, NT, E], f32)
            nc.vector.tensor_mul(msel, lgall, sel_sb[:, None, :].to_broadcast([P, NT, E]))
            le = asmp.tile([P, NT], f32)
            nc.vector.reduce_sum(le, msel, axis=AX.X)
            m1 = asmp.tile([P, NT], f32)
            nc.vector.tensor_tensor(m1, le, l1, op=OP.is_equal)
            m2 = asmp.tile([P, NT], f32)
            nc.vector.tensor_tensor(m2, le, l2, op=OP.is_equal)
            w_all = asmp.tile([P, NT], f32)
            t1 = asmp.tile([P, NT], f32)
            nc.vector.tensor_mul(t1, m1, w1a)
            t2 = asmp.tile([P, NT], f32)
            nc.vector.tensor_mul(t2, m2, w2a)
            nc.vector.tensor_add(w_all, t1, t2)
            m_all = asmp.tile([P, NT], f32)
            nc.vector.tensor_add(m_all, m1, m2)
            nc.vector.tensor_scalar_min(m_all, m_all, 1.0)

            # ---- exclusive cumsum of mask over all T tokens (t = c*128 + p) ----
            # column-transposed partial: S_exclT[c, p] = sum_{q<p} m[q, c]
            ps_se = psum_sm.tile([NT, P], f32, tag="small")
            nc.tensor.matmul(ps_se, lhsT=m_all, rhs=u128_sb, start=True, stop=True)
            # per-column totals -> exclusive column offsets
            ps_mt = psum_sm.tile([NT, P], f32, tag="small")
            nc.tensor.transpose(ps_mt, m_all, idf_sb)
            mt = asmp.tile([NT, P], f32)
            nc.vector.tensor_copy(mt, ps_mt)
            colsum = asmp.tile([NT, 1], f32)
            nc.vector.reduce_sum(colsum, mt, axis=AX.X)
            ps_off = psum_sm.tile([NT, 1], f32, tag="small")
            nc.tensor.matmul(ps_off, lhsT=u32_sb, rhs=colsum, start=True, stop=True)
            offs = asmp.tile([NT, 1], f32)
            nc.vector.tensor_copy(offs, ps_off)
            posT = asmp.tile([NT, P], f32)
            nc.vector.tensor_scalar_add(posT, ps_se, offs)
            ps_pos = psum_sm.tile([P, NT], f32, tag="small")
            nc.tensor.transpose(ps_pos, posT, idf_sb[:NT, :NT])
            # mask out unselected tokens -> huge position (skipped by bounds check)
            pm = asmp.tile([P, NT], f32)
            nc.vector.tensor_mul(pm, ps_pos, m_all)
            bigt = asmp.tile([P, NT], f32)
            nc.vector.tensor_scalar(bigt, m_all, -float(BIG), float(BIG),
                                    op0=OP.mult, op1=OP.add)
            nc.vector.tensor_add(pm, pm, bigt)
            pos_i = asmp.tile([P, NT], i32)
            nc.vector.tensor_copy(pos_i, pm)

            # scatter token rows directly into compact slots (merged gather)
            xb_re = xb_d.rearrange("(jo p) d -> p jo d", p=P)
            for jo in range(NT // 2):
                xb2 = gthp.tile([P, 2, D], bf, tag="xb2")
                nc.sync.dma_start(xb2, xb_re[:, jo * 2:(jo + 1) * 2, :])
                for jj in range(2):
                    c = jo * 2 + jj
                    nc.gpsimd.indirect_dma_start(
                        out=xgath,
                        out_offset=IndirectOffsetOnAxis(ap=pos_i[:, c:c + 1], axis=0),
                        in_=xb2[:, jj, :],
                        in_offset=None,
                        bounds_check=CAP - 1,
                        oob_is_err=False,
                    )

            # scatter (token_id, weight) to compact slots
            sc = asmp.tile([P, NT, 2], f32)
            nc.vector.tensor_copy(sc[:, :, 0], iota_sb)
            nc.vector.tensor_copy(sc[:, :, 1], w_all)
            with nc.allow_non_contiguous_dma(reason="8B-row compaction scatter"):
                for c in range(NT):
                    nc.gpsimd.indirect_dma_start(
                        out=meta_out,
                        out_offset=IndirectOffsetOnAxis(ap=pos_i[:, c:c + 1], axis=0),
                        in_=sc[:, c, :],
                        in_offset=None,
                        bounds_check=CAP - 1,
                        oob_is_err=False,
                    )

            # all scatters land before the compact slots are read back
            tc.strict_bb_all_engine_barrier()

            # ---- expert MLP over capacity tiles ----
            wv = asmp.tile([P, NJ], f32)   # per-slot routing weight
            for g, (t0, ng) in enumerate(GROUPS):
                njg = ng // P
                xtg = xgp.tile([P, KO, 512], bf, tag="xtgrp")
                for jj in range(njg):
                    j = t0 // P + jj
                    meta_t = gthp.tile([P, 2], f32, tag="meta")
                    with nc.allow_non_contiguous_dma(reason="8B meta rows"):
                        nc.sync.dma_start(meta_t, meta_out[j * P:(j + 1) * P, :])
                    nc.vector.tensor_copy(wv[:, j:j + 1], meta_t[:, 1:2])
                    xg = gthp.tile([P, D], bf, tag="xg")
                    nc.sync.dma_start(xg, xgath[j * P:(j + 1) * P, :])
                    for ko in range(KO):
                        ps_tr = psum_sm.tile([P, P], bf, tag="trb")
                        nc.tensor.transpose(ps_tr, xg[:, ko * P:(ko + 1) * P], idb_sb)
                        nc.vector.tensor_copy(xtg[:, ko, jj * P:(jj + 1) * P], ps_tr)

                # gate/up + silu + mul (act kept in gsil, bf16)
                gsil = mlpp.tile([P, KI, 512], bf, tag="gsil")
                for m in range(KI):
                    ps = psum_mm.tile([P, 512], f32, tag="mm")
                    for ko in range(KO):
                        nc.tensor.matmul(ps[:, :ng], lhsT=wg_sb[:, ko, m * P:(m + 1) * P],
                                         rhs=xtg[:, ko, :ng], start=(ko == 0),
                                         stop=(ko == KO - 1))
                    nc.scalar.activation(gsil[:, m, :ng], ps[:, :ng], AF.Sigmoid)
                    nc.vector.tensor_mul(gsil[:, m, :ng], ps[:, :ng], gsil[:, m, :ng])
                    ps2 = psum_mm.tile([P, 512], f32, tag="mm")
                    for ko in range(KO):
                        nc.tensor.matmul(ps2[:, :ng], lhsT=wi_sb[:, ko, m * P:(m + 1) * P],
                                         rhs=xtg[:, ko, :ng], start=(ko == 0),
                                         stop=(ko == KO - 1))
                    nc.vector.tensor_mul(gsil[:, m, :ng], ps2[:, :ng], gsil[:, m, :ng])

                # down proj + transpose back + per-token scale + store
                for do in range(KO):
                    ps3 = psum_mm.tile([P, 512], f32, tag="mm")
                    for ki in range(KI):
                        nc.tensor.matmul(ps3[:, :ng], lhsT=wo_sb[:, ki, do * P:(do + 1) * P],
                                         rhs=gsil[:, ki, :ng], start=(ki == 0),
                                         stop=(ki == KI - 1))
                    ysb = ytp.tile([P, 512], f32, tag="ysb")
                    nc.vector.tensor_copy(ysb[:, :ng], ps3[:, :ng])
                    for jj in range(njg):
                        j = t0 // P + jj
                        ps4 = psum_sm.tile([P, P], f32, tag="small")
                        nc.tensor.transpose(ps4, ysb[:, jj * P:(jj + 1) * P], idf_sb)
                        ystg = ytp.tile([P, P], f32, tag="ystg")
                        nc.vector.tensor_scalar_mul(ystg, ps4, wv[:, j:j + 1])
                        nc.sync.dma_start(
                            y_out[j * P:(j + 1) * P, do * P:(do + 1) * P], ystg)

    nc.compile()
    return nc


def _get_nc():
    if "nc" not in _CACHE:
        _CACHE["nc"] = _build_nc()
    return _CACHE["nc"]


def _make_in_maps(x, W_router, W_gate, W_in, W_out):
    bf16 = ml_dtypes.bfloat16
    x2d = np.ascontiguousarray(x.reshape(T, D).astype(np.float32))
    x_t = np.ascontiguousarray(x2d.T)
    x_bf = np.ascontiguousarray(x2d.astype(bf16))
    wr = np.ascontiguousarray(W_router.astype(np.float32))
    u128 = np.triu(np.ones((P, P), np.float32), 1)
    u32 = np.triu(np.ones((NT, NT), np.float32), 1)
    idf = np.eye(P, dtype=np.float32)
    idb = np.eye(P, dtype=np.float32).astype(bf16)
    iota = (np.arange(P, dtype=np.float32)[:, None]
            + P * np.arange(NT, dtype=np.float32)[None, :])
    iota = np.ascontiguousarray(iota.astype(np.float32))

    in_maps = []
    for e in range(N_CORES):
        sel = np.zeros((P, E), np.float32)
        sel[:, e] = 1.0
        in_maps.append({
            "x_t": x_t,
            "x_bf": x_bf,
            "w_r": wr,
            "w_g": np.ascontiguousarray(W_gate[e].astype(bf16)),
            "w_i": np.ascontiguousarray(W_in[e].astype(bf16)),
            "w_o": np.ascontiguousarray(W_out[e].astype(bf16)),
            "sel": sel,
            "u128": u128,
            "u32": u32,
            "idf": idf,
            "idb": idb,
            "iota": iota,
        })
    return in_maps


def kernel(x, W_router, W_gate, W_in, W_out, _trace=False, _trace_cores=None):
    from concourse.bass_utils import run_bass_kernel_spmd

    nc = _get_nc()
    in_maps = _make_in_maps(x, W_router, W_gate, W_in, W_out)
    res = run_bass_kernel_spmd(nc, in_maps, list(range(N_CORES)),
                               trace=_trace, trace_cores=_trace_cores)
    kernel._last_results = res

    logits = np.asarray(res.results[0]["logits_out"], dtype=np.float32)
    out = np.zeros((T, D), np.float32)
    for e in range(N_CORES):
        meta = np.asarray(res.results[e]["meta_out"])
        y = np.asarray(res.results[e]["y_out"])
        valid = meta[:, 1] != 0.0
        idx = meta[valid, 0].astype(np.int64)
        out[idx] += y[valid]
    B, S = 2, 2048
    return out.reshape(B, S, D), logits


# revision 18
# speedup vs baseline: 1.1246x; 1.1246x over previous
"""MoE (top-2 of 8 experts, D=1024, F=2048, T=4096) on 8 Trainium2 NeuronCores.

Strategy: expert-parallel. Every core replicates the fp32 router over all
4096 tokens, selects the tokens routed to ITS expert (top-2 membership via
max8 on logits; weights w1=sigmoid(l1-l2) renormalized pair weights),
compacts their indices with a matmul-based exclusive cumsum + indirect-DMA
scatter, gathers those token rows, runs the gated-MLP for its single expert
in bf16 (fp32 accumulate), scales each token's output row by its routing
weight, and writes a compact [CAP, D] fp32 result + the slot->token map.
The host sums the 8 compact shards into the full [T, D] output. Router
logits are computed in fp32 on-device and returned from core 0.
"""

import os
import sys

import numpy as np
import ml_dtypes

if "/opt/trn_rl_repo" not in sys.path:
    sys.path.insert(0, "/opt/trn_rl_repo")

# Problem shapes (hardcoded per contract)
T, D, F, E = 4096, 1024, 2048, 8
P = 128
NT = T // P            # 32 token tiles of 128
CAP = 1280             # per-expert token capacity (expected load 1024, ~9 sigma margin)
NJ = CAP // P          # 10 capacity tiles
GROUPS = [(0, 512), (512, 512), (1024, 256)]  # (token offset, group size) in CAP space
KO = D // P            # 8 contraction chunks over D
KI = F // P            # 16 contraction chunks over F
DP = D + 32            # compact row width, 64B-aligned (w_hi/w_lo at D, D+1)
BIG = 100000           # position sentinel for unselected tokens
                       # (BIG * row-stride must stay well inside int32)

N_CORES = 8

_CACHE = {}


def _build_nc():
    import concourse.tile as tile
    from concourse import bacc, mybir
    from concourse.bass import IndirectOffsetOnAxis

    f32 = mybir.dt.float32
    bf = mybir.dt.bfloat16
    i32 = mybir.dt.int32
    AF = mybir.ActivationFunctionType
    AX = mybir.AxisListType
    OP = mybir.AluOpType

    nc = bacc.Bacc("TRN2", target_bir_lowering=False, debug=False,
                   enable_asserts=False, num_devices=N_CORES)

    # ---- I/O ----
    xt_d = nc.dram_tensor("x_t", [D, T], f32, kind="ExternalInput").ap()
    xb_d = nc.dram_tensor("x_bf", [T, D], bf, kind="ExternalInput").ap()
    wr_d = nc.dram_tensor("w_r", [D, E], f32, kind="ExternalInput").ap()
    wg_d = nc.dram_tensor("w_g", [D, F], bf, kind="ExternalInput").ap()
    wi_d = nc.dram_tensor("w_i", [D, F], bf, kind="ExternalInput").ap()
    wo_d = nc.dram_tensor("w_o", [F, D], bf, kind="ExternalInput").ap()
    sel_d = nc.dram_tensor("sel", [P, E], f32, kind="ExternalInput").ap()
    u128_d = nc.dram_tensor("u128", [P, P], f32, kind="ExternalInput").ap()
    u32_d = nc.dram_tensor("u32", [NT, NT], f32, kind="ExternalInput").ap()
    idf_d = nc.dram_tensor("idf", [P, P], f32, kind="ExternalInput").ap()
    idb_d = nc.dram_tensor("idb", [P, P], bf, kind="ExternalInput").ap()

    lg_out = nc.dram_tensor("logits_out", [T, E], f32, kind="ExternalOutput").ap()
    y_out = nc.dram_tensor("y_out", [CAP, D], f32, kind="ExternalOutput").ap()
    xgath = nc.dram_tensor("xgath", [CAP, DP], bf).ap()  # row = [x, w_hi, w_lo, pad]

    with tile.TileContext(nc) as tc:
        from contextlib import ExitStack
        with ExitStack() as ctx:
            consts = ctx.enter_context(tc.tile_pool(name="consts", bufs=1))
            wpool = ctx.enter_context(tc.tile_pool(name="wpool", bufs=1))
            xtp = ctx.enter_context(tc.tile_pool(name="xtp", bufs=3))
            rsm = ctx.enter_context(tc.tile_pool(name="rsm", bufs=4))
            asmp = ctx.enter_context(tc.tile_pool(name="asm", bufs=1))
            gthp = ctx.enter_context(tc.tile_pool(name="gth", bufs=2))
            xgp = ctx.enter_context(tc.tile_pool(name="xgp", bufs=2))
            mlpp = ctx.enter_context(tc.tile_pool(name="mlp", bufs=2))
            ytp = ctx.enter_context(tc.tile_pool(name="ytp", bufs=2))
            psum_mm = ctx.enter_context(tc.tile_pool(name="psmm", bufs=4, space="PSUM"))
            psum_sm = ctx.enter_context(tc.tile_pool(name="pssm", bufs=2, space="PSUM"))

            # ---- constants ----
            wr_sb = consts.tile([P, KO, E], f32)
            nc.sync.dma_start(wr_sb, wr_d.rearrange("(ko p) e -> p ko e", p=P))
            sel_sb = consts.tile([P, E], f32)
            nc.sync.dma_start(sel_sb, sel_d)
            u128_sb = consts.tile([P, P], f32)
            nc.sync.dma_start(u128_sb, u128_d)
            u32_sb = consts.tile([NT, NT], f32)
            nc.sync.dma_start(u32_sb, u32_d)
            idf_sb = consts.tile([P, P], f32)
            nc.sync.dma_start(idf_sb, idf_d)
            idb_sb = consts.tile([P, P], bf)
            nc.sync.dma_start(idb_sb, idb_d)

            # ---- router: logits for all tokens, fp32 ----
            # logitsT[e, t-chunk] = W_router.T @ x (tiny stationary operand),
            # then PE-transpose each 128-token chunk into [t, E] layout.
            lgall = asmp.tile([P, NT, E], f32)    # logits, token t = c*128 + p
            l12 = asmp.tile([P, NT, 8], f32)      # max8 sorted logits per tile
            xt_re = xt_d.rearrange("(ko p) t -> p ko t", p=P)
            for tg in range(T // 256):
                xt_g = xtp.tile([P, KO, 256], f32, tag="xtg")
                nc.sync.dma_start(xt_g, xt_re[:, :, tg * 256:(tg + 1) * 256])
                ps_lt = psum_sm.tile([E, 256], f32, tag="small")
                for ko in range(KO):
                    nc.tensor.matmul(ps_lt, lhsT=wr_sb[:, ko], rhs=xt_g[:, ko],
                                     start=(ko == 0), stop=(ko == KO - 1))
                lgt = rsm.tile([E, 256], f32, tag="lgt")
                nc.vector.tensor_copy(lgt, ps_lt)
                for jj in range(2):
                    j = tg * 2 + jj
                    ps_l = psum_sm.tile([P, E], f32, tag="small")
                    nc.tensor.transpose(ps_l, lgt[:, jj * P:(jj + 1) * P],
                                        idf_sb[:E, :E])
                    nc.vector.tensor_copy(lgall[:, j], ps_l)
                    nc.vector.max(l12[:, j], lgall[:, j])

            # logits output (token-major [T, E])
            nc.sync.dma_start(lg_out.rearrange("(c p) e -> p c e", p=P), lgall)

            # ---- expert weight preload (bf16, stays resident). Issued after
            # the router reads so the big transfers don't delay them. ----
            wg_sb = wpool.tile([P, KO, F], bf)
            nc.sync.dma_start(wg_sb, wg_d.rearrange("(ko p) f -> p ko f", p=P))
            wi_sb = wpool.tile([P, KO, F], bf)
            nc.sync.dma_start(wi_sb, wi_d.rearrange("(ko p) f -> p ko f", p=P))
            wo_sb = wpool.tile([P, KI, D], bf)
            nc.sync.dma_start(wo_sb, wo_d.rearrange("(ki p) d -> p ki d", p=P))

            # zero-init the compacted-row buffer (junk rows -> x=0, w=0)
            xz = asmp.tile([P, DP], bf)
            nc.vector.memset(xz, 0.0)
            for j in range(NJ):
                nc.sync.dma_start(xgath[j * P:(j + 1) * P, :], xz)

            # ---- top-2 weights + this-expert membership, batched ----
            l1 = l12[:, :, 0]
            l2 = l12[:, :, 1]
            d21 = asmp.tile([P, NT], f32)
            nc.vector.tensor_sub(d21, l2, l1)
            w2a = asmp.tile([P, NT], f32)
            nc.scalar.activation(w2a, d21, AF.Sigmoid)      # w2 = sigmoid(l2 - l1)
            w1a = asmp.tile([P, NT], f32)
            nc.vector.tensor_scalar(w1a, w2a, -1.0, 1.0, op0=OP.mult, op1=OP.add)
            msel = asmp.tile([P, NT, E], f32)
            nc.vector.tensor_mul(msel, lgall, sel_sb[:, None, :].to_broadcast([P, NT, E]))
            le = asmp.tile([P, NT], f32)
            nc.vector.reduce_sum(le, msel, axis=AX.X)
            m1 = asmp.tile([P, NT], f32)
            nc.vector.tensor_tensor(m1, le, l1, op=OP.is_equal)
            m2 = asmp.tile([P, NT], f32)
            nc.vector.tensor_tensor(m2, le, l2, op=OP.is_equal)
            w_all = asmp.tile([P, NT], f32)
            t1 = asmp.tile([P, NT], f32)
            nc.vector.tensor_mul(t1, m1, w1a)
            t2 = asmp.tile([P, NT], f32)
            nc.vector.tensor_mul(t2, m2, w2a)
            nc.vector.tensor_add(w_all, t1, t2)
            m_all = asmp.tile([P, NT], f32)
            nc.vector.tensor_add(m_all, m1, m2)
            nc.vector.tensor_scalar_min(m_all, m_all, 1.0)

            # ---- exclusive cumsum of mask over all T tokens (t = c*128 + p) ----
            # column-transposed partial: S_exclT[c, p] = sum_{q<p} m[q, c]
            ps_se = psum_sm.tile([NT, P], f32, tag="small")
            nc.tensor.matmul(ps_se, lhsT=m_all, rhs=u128_sb, start=True, stop=True)
            # per-column totals -> exclusive column offsets
            ps_mt = psum_sm.tile([NT, P], f32, tag="small")
            nc.tensor.transpose(ps_mt, m_all, idf_sb)
            mt = asmp.tile([NT, P], f32)
            nc.vector.tensor_copy(mt, ps_mt)
            colsum = asmp.tile([NT, 1], f32)
            nc.vector.reduce_sum(colsum, mt, axis=AX.X)
            ps_off = psum_sm.tile([NT, 1], f32, tag="small")
            nc.tensor.matmul(ps_off, lhsT=u32_sb, rhs=colsum, start=True, stop=True)
            offs = asmp.tile([NT, 1], f32)
            nc.vector.tensor_copy(offs, ps_off)
            posT = asmp.tile([NT, P], f32)
            nc.vector.tensor_scalar_add(posT, ps_se, offs)
            ps_pos = psum_sm.tile([P, NT], f32, tag="small")
            nc.tensor.transpose(ps_pos, posT, idf_sb[:NT, :NT])
            # mask out unselected tokens -> huge position (skipped by bounds check)
            pm = asmp.tile([P, NT], f32)
            nc.vector.tensor_mul(pm, ps_pos, m_all)
            bigt = asmp.tile([P, NT], f32)
            nc.vector.tensor_scalar(bigt, m_all, -float(BIG), float(BIG),
                                    op0=OP.mult, op1=OP.add)
            nc.vector.tensor_add(pm, pm, bigt)
            pos_i = asmp.tile([P, NT], i32)
            nc.vector.tensor_copy(pos_i, pm)

            # scatter token rows (with embedded split-bf16 routing weight)
            # directly into compact slots -- the gather is merged into this.
            xb_re = xb_d.rearrange("(jo p) d -> p jo d", p=P)
            for jo in range(NT // 2):
                xb2 = gthp.tile([P, 2, DP], bf, tag="xb2")
                nc.vector.memset(xb2[:, :, D + 2:], 0.0)
                nc.sync.dma_start(xb2[:, :, :D], xb_re[:, jo * 2:(jo + 1) * 2, :])
                for jj in range(2):
                    c = jo * 2 + jj
                    nc.vector.tensor_copy(xb2[:, jj, D:D + 1], w_all[:, c:c + 1])
                    wlo = rsm.tile([P, 1], f32, tag="wlo")
                    nc.vector.tensor_sub(wlo, w_all[:, c:c + 1], xb2[:, jj, D:D + 1])
                    nc.vector.tensor_copy(xb2[:, jj, D + 1:D + 2], wlo)
                    nc.gpsimd.indirect_dma_start(
                        out=xgath,
                        out_offset=IndirectOffsetOnAxis(ap=pos_i[:, c:c + 1], axis=0),
                        in_=xb2[:, jj, :],
                        in_offset=None,
                        bounds_check=CAP - 1,
                        oob_is_err=False,
                    )

            # all scatters land before the compact slots are read back
            tc.strict_bb_all_engine_barrier()

            # ---- expert MLP over capacity tiles ----
            wv = asmp.tile([P, NJ], f32)   # per-slot routing weight
            for g, (t0, ng) in enumerate(GROUPS):
                njg = ng // P
                xtg = xgp.tile([P, KO, 512], bf, tag="xtgrp")
                for jj in range(njg):
                    j = t0 // P + jj
                    xg = gthp.tile([P, DP], bf, tag="xg")
                    nc.sync.dma_start(xg, xgath[j * P:(j + 1) * P, :])
                    nc.vector.tensor_tensor(wv[:, j:j + 1], xg[:, D:D + 1],
                                            xg[:, D + 1:D + 2], op=OP.add)
                    for ko in range(KO):
                        ps_tr = psum_sm.tile([P, P], bf, tag="trb")
                        nc.tensor.transpose(ps_tr, xg[:, ko * P:(ko + 1) * P], idb_sb)
                        nc.vector.tensor_copy(xtg[:, ko, jj * P:(jj + 1) * P], ps_tr)

                # gate/up + silu + mul (act kept in gsil, bf16)
                gsil = mlpp.tile([P, KI, 512], bf, tag="gsil")
                for m in range(KI):
                    ps = psum_mm.tile([P, 512], f32, tag="mm")
                    for ko in range(KO):
                        nc.tensor.matmul(ps[:, :ng], lhsT=wg_sb[:, ko, m * P:(m + 1) * P],
                                         rhs=xtg[:, ko, :ng], start=(ko == 0),
                                         stop=(ko == KO - 1))
                    nc.scalar.activation(gsil[:, m, :ng], ps[:, :ng], AF.Sigmoid)
                    nc.vector.tensor_mul(gsil[:, m, :ng], ps[:, :ng], gsil[:, m, :ng])
                    ps2 = psum_mm.tile([P, 512], f32, tag="mm")
                    for ko in range(KO):
                        nc.tensor.matmul(ps2[:, :ng], lhsT=wi_sb[:, ko, m * P:(m + 1) * P],
                                         rhs=xtg[:, ko, :ng], start=(ko == 0),
                                         stop=(ko == KO - 1))
                    nc.vector.tensor_mul(gsil[:, m, :ng], ps2[:, :ng], gsil[:, m, :ng])

                # down proj + transpose back + per-token scale + store
                for do in range(KO):
                    ps3 = psum_mm.tile([P, 512], f32, tag="mm")
                    for ki in range(KI):
                        nc.tensor.matmul(ps3[:, :ng], lhsT=wo_sb[:, ki, do * P:(do + 1) * P],
                                         rhs=gsil[:, ki, :ng], start=(ki == 0),
                                         stop=(ki == KI - 1))
                    ysb = ytp.tile([P, 512], f32, tag="ysb")
                    nc.vector.tensor_copy(ysb[:, :ng], ps3[:, :ng])
                    for jj in range(njg):
                        j = t0 // P + jj
                        ps4 = psum_sm.tile([P, P], f32, tag="small")
                        nc.tensor.transpose(ps4, ysb[:, jj * P:(jj + 1) * P], idf_sb)
                        ystg = ytp.tile([P, P], f32, tag="ystg")
                        nc.vector.tensor_scalar_mul(ystg, ps4, wv[:, j:j + 1])
                        nc.sync.dma_start(
                            y_out[j * P:(j + 1) * P, do * P:(do + 1) * P], ystg)

    nc.compile()
    return nc


def _get_nc():
    if "nc" not in _CACHE:
        _CACHE["nc"] = _build_nc()
    return _CACHE["nc"]


def _make_in_maps(x, W_router, W_gate, W_in, W_out):
    bf16 = ml_dtypes.bfloat16
    x2d = np.ascontiguousarray(x.reshape(T, D).astype(np.float32))
    x_t = np.ascontiguousarray(x2d.T)
    x_bf = np.ascontiguousarray(x2d.astype(bf16))
    wr = np.ascontiguousarray(W_router.astype(np.float32))
    u128 = np.triu(np.ones((P, P), np.float32), 1)
    u32 = np.triu(np.ones((NT, NT), np.float32), 1)
    idf = np.eye(P, dtype=np.float32)
    idb = np.eye(P, dtype=np.float32).astype(bf16)

    in_maps = []
    for e in range(N_CORES):
        sel = np.zeros((P, E), np.float32)
        sel[:, e] = 1.0
        in_maps.append({
            "x_t": x_t,
            "x_bf": x_bf,
            "w_r": wr,
            "w_g": np.ascontiguousarray(W_gate[e].astype(bf16)),
            "w_i": np.ascontiguousarray(W_in[e].astype(bf16)),
            "w_o": np.ascontiguousarray(W_out[e].astype(bf16)),
            "sel": sel,
            "u128": u128,
            "u32": u32,
            "idf": idf,
            "idb": idb,
        })
    return in_maps


def kernel(x, W_router, W_gate, W_in, W_out, _trace=False, _trace_cores=None):
    from concourse.bass_utils import run_bass_kernel_spmd

    nc = _get_nc()
    in_maps = _make_in_maps(x, W_router, W_gate, W_in, W_out)
    res = run_bass_kernel_spmd(nc, in_maps, list(range(N_CORES)),
                               trace=_trace, trace_cores=_trace_cores)
    kernel._last_results = res

    logits = np.asarray(res.results[0]["logits_out"], dtype=np.float32)
    # Replicate the device's top-2 membership mask bit-exactly from the same
    # fp32 logits the device routed with; slot order == ascending token id.
    srt = np.sort(logits, axis=1)[:, ::-1]
    l1, l2 = srt[:, 0:1], srt[:, 1:2]
    member = (logits == l1) | (logits == l2)      # [T, E]
    out = np.zeros((T, D), np.float32)
    for e in range(N_CORES):
        y = np.asarray(res.results[e]["y_out"])
        idx = np.nonzero(member[:, e])[0][:CAP]
        out[idx] += y[:len(idx)]
    B, S = 2, 2048
    return out.reshape(B, S, D), logits
